# revision 5
# baseline (speedup 1.0000x reference)
"""Trainium2 Bass kernel for the 9-layer dense MLP (dropout-mask training forward).

Strategy (pure data parallel, 8 cores, 8192 batch rows each):
  - Activations kept transposed on-chip: features on partitions, batch cols on free dim.
    Each layer computes zT = W^T @ hT via nc.tensor.matmul(out, lhsT=W, rhs=hT).
  - fp16 weights/activations/masks (fp32 PSUM accumulation), fp32 biases + output.
  - Dropout masks binarized on host ({0,1} fp16); the 1/keep scale is folded into the
    next layer's weights.
  - Host pack layout [NBLK, 128, NPACK, BLK] so each per-block DMA is contiguous per
    partition (2 DMAs per block: x+m1, then the remaining masks).
  - PSUM: one shared pool of 4x [128,1024] fp32 tiles (8 banks). Matmuls write 512-col
    halves; drains are single FD=1024 instructions (fused bias+relu) split ~3:1
    ACT:DVE; mask multiplies are FD=2048 DVE tensor_tensor with a measured dose
    offloaded to GpSimd.
  - Small layers 6/7/8 partition-packed (offsets 0/64/96 via matmul tile_position);
    each ladder step drains immediately to SBUF so its PSUM tile recycles fast.
    Block b's ladder is software-pipelined into block b+1's big-layer bursts.
  - A short burst of dummy matmuls at t~1us keeps the PE HAM activity monitor busy so
    the array is at full clock (K=8/8) when real work arrives.
"""

import sys

sys.path.insert(0, "/opt/trn_rl_repo")

import numpy as np

DIMS = [256, 128, 256, 512, 256, 128, 64, 32, 16, 10]
NCORES = 8
BATCH = 65536
SHARD = BATCH // NCORES  # 8192
MSUB = 512               # matmul N (PSUM bank limit for fp32)
DSUB = 1024              # drain granularity (2 banks)
BLK = 2048               # block columns
NBLK = SHARD // BLK      # 4
NSUB = BLK // MSUB       # 4
NDR = BLK // DSUB        # 2

# pack chunk layout (each chunk = 128 partitions x BLK cols, fp16), per block:
#   0,1: xT        2: m1        3,4: m2      5-8: m3
#   9,10: m4       11: m5       12: m6/m7/m8 partition-packed at rows 0/64/96
NPACK = 13

_PROG = {}


def _raise_sbuf_cap():
    # tile_utils.max_sbuf_usage is a stale 192KB constant; cayman has 208KB usable.
    import concourse.tile_utils as tu

    if getattr(tu, "max_sbuf_usage", 0) < 206 * 1024:
        tu.max_sbuf_usage = 206 * 1024


def _dedup_ldweights(nc):
    """Remove back-to-back redundant LDWEIGHTS (same stationary operand) so
    consecutive same-weight matmuls pipeline on the PE. Only drops LDW
    instructions that carry no semaphore waits/updates."""
    removed = 0
    for fn in nc.m.functions:
        for blk in fn.blocks:
            il = blk.instructions
            keep, last_sig = [], None
            for inst in il:
                nm = type(inst).__name__
                if nm == "InstLdweights":
                    sig = (str(inst.ins[0]), str(inst.is_transpose), str(inst.perf_mode),
                           str(getattr(inst, "tile_position", None)))
                    si = inst.sync_info
                    clean = si is None or (not si.on_wait and not si.on_update)
                    if sig == last_sig and clean:
                        removed += 1
                        continue
                    last_sig = sig
                keep.append(inst)
            if removed and len(keep) != len(il):
                while il:
                    il.pop()
                il.extend(keep)
    return removed


def _build_program():
    import concourse.bass as bass
    import concourse.tile as tile
    from concourse import bacc, mybir

    _raise_sbuf_cap()

    f16 = mybir.dt.float16
    f32 = mybir.dt.float32
    RELU = mybir.ActivationFunctionType.Relu
    IDENT = mybir.ActivationFunctionType.Identity
    ADD = mybir.AluOpType.add
    MAX = mybir.AluOpType.max

    nc = bacc.Bacc("TRN2", target_bir_lowering=False, debug=False, num_devices=NCORES)

    pack_d = nc.dram_tensor("pack", [NBLK, 128, NPACK, BLK], f16, kind="ExternalInput").ap()
    # weights in two host-laid-out fp16 blobs (W1 separate so the first
    # LDWEIGHTS isn't gated on the full blob), biases in one fp32 blob
    wb1_d = nc.dram_tensor("WB1", [128, 256], f16, kind="ExternalInput").ap()
    wb_d = nc.dram_tensor("WB", [128, 2688], f16, kind="ExternalInput").ap()
    bb_d = nc.dram_tensor("BB", [128, 12], f32, kind="ExternalInput").ap()
    out_d = nc.dram_tensor("outT", [10, SHARD], f32, kind="ExternalOutput").ap()

    with tile.TileContext(nc) as tc:
        with (
            tc.tile_pool(name="wpool", bufs=1) as wp,
            tc.tile_pool(name="mk", bufs=2) as mkp,
            tc.tile_pool(name="hr", bufs=1) as hrp,
            tc.tile_pool(name="osb", bufs=2) as outp,
            tc.tile_pool(name="ps", bufs=4, space="PSUM") as psp,
        ):
            wall = wp.tile([128, 2944], f16, tag="wall")
            ball = wp.tile([128, 12], f32, tag="ball")
            scratch = wp.tile([128, 512], f16, tag="scratch")
            # blob column offsets: w1@0(256) w2@256(256) w3@512(1024) w4@1536(1024)
            #   w5@2560(256) w6@2816(64) w789@2880(64: W7 r0-63 c0-31, W8 r64-95
            #   c32-47, W9 r96-111 c48-57)
            WOFF = {1: 0, 2: 256, 3: 512, 4: 1536, 5: 2560, 6: 2816, 789: 2880}
            w789 = wall[:, WOFF[789]:WOFF[789] + 64]
            b15 = ball[:, 0:10]
            b678 = ball[:, 10:11]
            b9 = ball[0:10, 11:12]

            def wslice(l, k, c, N):
                base = WOFF[l] + k * N
                return wall[:, base + c * 128: base + (c + 1) * 128]

            def drain_relu(eng, dst, zsrc, bias_ap):
                if eng == "act":
                    nc.scalar.activation(dst, zsrc, RELU, bias=bias_ap)
                else:
                    nc.vector.tensor_scalar(dst, zsrc, bias_ap, 0.0, ADD, MAX)

            def mask_mul(eng, dst, src, msrc):
                if eng == "gps":
                    nc.gpsimd.tensor_mul(dst, src, msrc)
                else:
                    nc.vector.tensor_mul(dst, src, msrc)

            # drain engine picker: ~3:1 act:dve
            dr_i = [0]

            def pick_drain():
                i = dr_i[0]
                dr_i[0] += 1
                return "dve" if i % 4 == 3 else "act"

            state = {}
            packs = {}
            hrs = {}

            def issue_pack_dmas(b):
                pk3 = mkp.tile([128, 3, BLK], f16, tag="pk3", name=f"pk3_{b}")
                mrest = mkp.tile([128, 9, BLK], f16, tag="mrest", name=f"mrest_{b}")
                # m678 is read by block b's ladder during block b+1, so bufs=3
                # keeps block b+2's DMA from waiting on the ladder (WAR).
                m678t = mkp.tile([128, 1, BLK], f16, tag="m678", name=f"m678_{b}",
                                 bufs=3)
                if b == 0:
                    # block 0: finer-grained so each layer's mask arrives just-in-time
                    nc.sync.dma_start(wall[:, 0:256], wb1_d[:])
                    nc.sync.dma_start(pk3[:, 0:2], pack_d[0, :, 0:2, :])
                    nc.sync.dma_start(pk3[:, 2:3], pack_d[0, :, 2:3, :])
                    nc.sync.dma_start(ball[:], bb_d[:])
                    nc.sync.dma_start(wall[:, 256:], wb_d[:])
                    nc.sync.dma_start(mrest[:, 0:2], pack_d[0, :, 3:5, :])
                    nc.sync.dma_start(mrest[:, 2:6], pack_d[0, :, 5:9, :])
                    nc.sync.dma_start(mrest[:, 6:9], pack_d[0, :, 9:12, :])
                    nc.sync.dma_start(m678t[:], pack_d[0, :, 12:13, :])
                else:
                    nc.sync.dma_start(pk3[:], pack_d[b, :, 0:3, :])
                    nc.sync.dma_start(mrest[:], pack_d[b, :, 3:12, :])
                    nc.sync.dma_start(m678t[:], pack_d[b, :, 12:13, :])
                packs[b] = (pk3, mrest, m678t)

            # mask chunk base offset within mrest, per big-layer index
            MOFF = {1: 0, 2: 2, 3: 6, 4: 8}

            # (Kc, layer, wN, Cc, bias_off, hrtag)
            LAYER_CFG = [
                (2, 1, 128, 1, 0, "hr1"),
                (1, 2, 256, 2, 1, "hr2"),
                (2, 3, 512, 4, 3, "hr3"),
                (4, 4, 256, 2, 7, "hr4"),
                (2, 5, 128, 1, 9, "hr5"),
            ]
            # mask engine schedule per layer index: list of (chunk c, mode) where
            # mode "dve" = one FD2048 DVE mul, "split" = DVE low half + GPS high half
            MASK_MODE = {
                0: ["dve"],
                1: ["dve", "split"],
                2: ["dve", "split", "dve", "split"],
                3: ["dve", "dve"],
                4: ["dve"],
            }

            def emit_layer(b, li):
                Kc, wl, wN, Cc, boff, hrtag = LAYER_CFG[li]
                hr = hrp.tile([128, Cc, BLK], f16, tag=hrtag, name=hrtag + f"_{b}",
                              bufs=2 if hrtag in ("hr5", "hr2", "hr1") else 1)
                pk3, mrest, _ = packs[b]
                hin = pk3 if li == 0 else hrs[(b, li - 1)]

                def msl(c, cols):
                    if li == 0:
                        return pk3[:, 2, cols]
                    return mrest[:, MOFF[li] + c, cols]

                zs = {}
                for c in range(Cc):
                    for u in range(NDR):
                        zs[c, u] = psp.tile([128, DSUB], f32, tag="ps",
                                            name=f"z_{hrtag}_{b}_{c}_{u}")
                if b == 0 and li == 0:
                    # startup: u-outer so each 1024-col window flows MM -> drain
                    # -> mask as early as possible
                    for u in range(NDR):
                        for k in range(Kc):
                            for t in range(2):
                                nc.tensor.matmul(
                                    zs[0, u][:, bass.ts(t, MSUB)],
                                    wslice(wl, k, 0, wN),
                                    hin[:, k, u * DSUB + t * MSUB:
                                        u * DSUB + (t + 1) * MSUB],
                                    start=(k == 0), stop=(k == Kc - 1))
                        drain_relu("dve" if u == 0 else "act",
                                   hr[:, 0, bass.ts(u, DSUB)], zs[0, u][:],
                                   b15[:, 0:1])
                        mask_mul("dve", hr[:, 0, bass.ts(u, DSUB)],
                                 hr[:, 0, bass.ts(u, DSUB)],
                                 msl(0, bass.ts(u, DSUB)))
                    hrs[(b, li)] = hr
                    return
                # weight-major matmuls so consecutive MMs share one LDWEIGHTS
                for c in range(Cc):
                    for k in range(Kc):
                        wap = wslice(wl, k, c, wN)
                        for t in range(NSUB):
                            nc.tensor.matmul(
                                zs[c, t // 2][:, bass.ts(t % 2, MSUB)], wap,
                                hin[:, k, bass.ts(t, MSUB)],
                                start=(k == 0), stop=(k == Kc - 1))
                for u in range(NDR):
                    for c in range(Cc):
                        drain_relu(pick_drain(), hr[:, c, bass.ts(u, DSUB)],
                                   zs[c, u][:], b15[:, boff + c:boff + c + 1])
                full = slice(0, BLK)
                for c in range(Cc):
                    mode = MASK_MODE[li][c]
                    if mode == "dve":
                        mask_mul("dve", hr[:, c, full], hr[:, c, full], msl(c, full))
                    else:
                        hs0, hs1 = bass.ts(0, DSUB), bass.ts(1, DSUB)
                        mask_mul("dve", hr[:, c, hs0], hr[:, c, hs0], msl(c, hs0))
                        mask_mul("gps", hr[:, c, hs1], hr[:, c, hs1], msl(c, hs1))
                hrs[(b, li)] = hr
                if li > 0:
                    del hrs[(b, li - 1)]

            # --- small-layer ladder (L6/L7/L8), partition-packed -----------------
            # step s writes partition range p0:p1 of hr678; fresh PSUM tiles per
            # step, drained immediately so banks recycle.
            LAD_CFG = [
                ((0, 64), None, None),     # L6: full-K weight slice, no tile_position
                ((64, 96), (0, 64), (0, 64)),
                ((96, 112), (64, 96), (64, 96)),
            ]

            def emit_ladder_step(b, step, fine=False):
                st = state[b]
                hm5, m678 = st["hm5"], st["m678"]
                if step == 0:
                    st["hr678"] = hrp.tile([128, 1, BLK], f16, tag="hr678",
                                           name=f"hr678_{b}", bufs=2)
                hr678 = st["hr678"]
                (p0, p1), tile_pos, brange = LAD_CFG[step]
                if step == 0:
                    wap = wall[:, WOFF[6]:WOFF[6] + 64]
                elif step == 1:
                    wap = w789[0:64, 0:32]
                else:
                    wap = w789[64:96, 32:48]
                wins = range(NSUB) if fine else range(NDR)
                wsz = MSUB if fine else DSUB
                for u in wins:
                    zh = psp.tile([128, wsz], f32, tag="ps", name=f"zl_{b}_{step}_{u}")
                    nmm = 1 if fine else 2
                    for t in range(nmm):
                        rhs_sl = slice(u * wsz + t * MSUB, u * wsz + (t + 1) * MSUB)
                        rhs = (hm5[:, 0, rhs_sl] if step == 0 else
                               hr678[brange[0]:brange[1], 0, rhs_sl])
                        out_sl = zh[p0:p1, bass.ts(t, MSUB)] if not fine else zh[p0:p1, :]
                        if tile_pos is None:
                            nc.tensor.matmul(out_sl, wap, rhs, start=True, stop=True)
                        else:
                            nc.tensor.matmul(out_sl, wap, rhs, start=True, stop=True,
                                             tile_position=tile_pos)
                    dst = hr678[p0:p1, 0, u * wsz:(u + 1) * wsz]
                    drain_relu("dve" if u % 2 == 0 else "act",
                               dst, zh[p0:p1, :], b678[p0:p1, 0:1])
                    mask_mul("gps" if (step == 0 and not fine and u == 1) else "dve",
                             dst, dst, m678[p0:p1, 0, u * wsz:(u + 1) * wsz])

            def emit_l9(b, fine=False):
                st = state[b]
                hm678 = st["hr678"]
                osb = outp.tile([10, BLK], f32, tag="osb", bufs=2, name=f"osb_{b}")
                wins = range(NSUB) if fine else range(NDR)
                wsz = MSUB if fine else DSUB
                for u in wins:
                    z9 = psp.tile([128, wsz], f32, tag="ps", name=f"z9_{b}_{u}")
                    nmm = 1 if fine else 2
                    for t in range(nmm):
                        rhs_sl = slice(u * wsz + t * MSUB, u * wsz + (t + 1) * MSUB)
                        nc.tensor.matmul(z9[0:10, bass.ts(t, MSUB)] if not fine
                                         else z9[0:10, :],
                                         w789[96:112, 48:58],
                                         hm678[96:112, 0, rhs_sl],
                                         start=True, stop=True, tile_position=(96, 0))
                    nc.scalar.activation(osb[:, u * wsz:(u + 1) * wsz], z9[0:10, :],
                                         IDENT, bias=b9[:, 0:1])
                nc.sync.dma_start(out_d[:, bass.ts(b, BLK)], osb[:])
                del state[b]

            # --- schedule ---------------------------------------------------------
            # PE warmup: dummy matmuls on a memset scratch tile so the HAM clock
            # gate opens before real work arrives (weights/x still in DMA).
            nc.vector.memset(scratch[:], 0.0)
            zw = psp.tile([128, MSUB], f32, tag="ps", name="zwarm")
            for i in range(8):
                nc.tensor.matmul(zw[:], scratch[:, 0:128], scratch[:], start=True,
                                 stop=True)

            issue_pack_dmas(0)
            emit_layer(0, 0)                       # L1(0) during startup
            for b in range(NBLK):
                if b + 1 < NBLK:
                    issue_pack_dmas(b + 1)
                emit_layer(b, 1)                   # L2
                if b > 0 and (b - 1) in state:
                    emit_ladder_step(b - 1, 0)     # L6(b-1)
                if b > 1 and (b - 2) in state:
                    emit_l9(b - 2)                 # L9(b-2)
                emit_layer(b, 2)                   # L3
                if b > 0 and (b - 1) in state:
                    emit_ladder_step(b - 1, 1)     # L7(b-1)
                emit_layer(b, 3)                   # L4
                if b + 1 < NBLK:
                    emit_layer(b + 1, 0)           # L1(b+1) pipelined ahead
                if b > 0 and (b - 1) in state:
                    emit_ladder_step(b - 1, 2)     # L8(b-1)
                emit_layer(b, 4)                   # L5
                state[b] = {"hm5": hrs.pop((b, 4)), "m678": packs[b][2]}

            # tail: last block's ladder at fine (512) granularity to shorten the
            # exposed serial chain; block NBLK-2's pending L9 fills the first gap
            emit_ladder_step(NBLK - 1, 0, fine=True)
            if (NBLK - 2) in state:
                emit_l9(NBLK - 2)
            emit_ladder_step(NBLK - 1, 1, fine=True)
            emit_ladder_step(NBLK - 1, 2, fine=True)
            emit_l9(NBLK - 1, fine=True)

    _dedup_ldweights(nc)
    nc.compile()
    return nc


def _get_program():
    if "nc" not in _PROG:
        _PROG["nc"] = _build_program()
    return _PROG["nc"]


def _host_prep(inputs):
    """Build per-core input maps (numpy only)."""
    x = np.asarray(inputs["x"], dtype=np.float32)
    Ws = [np.asarray(inputs[f"W{i}"], dtype=np.float32) for i in range(1, 10)]
    bs = [np.asarray(inputs[f"b{i}"], dtype=np.float32) for i in range(1, 10)]
    ms = [np.asarray(inputs[f"m{i}"], dtype=np.float32) for i in range(1, 9)]

    # fold dropout scale into next layer's weights; binarize masks
    Wf = [Ws[0]]
    for i in range(1, 9):
        s = float(ms[i - 1].max())
        if s <= 0.0:  # degenerate all-dropped mask; keep weights unscaled
            s = 1.0
        Wf.append(Ws[i] * np.float32(s))

    # weight blob: w1@0 w2@256 w3@512 w4@1536 w5@2560 w6@2816 w789@2880
    WOFF = {1: 0, 2: 256, 3: 512, 4: 1536, 5: 2560, 6: 2816, 789: 2880}
    wb = np.zeros((128, 2944), dtype=np.float16)
    for l in range(1, 7):
        W = Wf[l - 1]
        K, N = W.shape
        for k in range((K + 127) // 128):
            blk = W[k * 128:(k + 1) * 128].astype(np.float16)
            wb[: blk.shape[0], WOFF[l] + k * N: WOFF[l] + k * N + N] = blk
    wb[0:64, 2880:2912] = Wf[6].astype(np.float16)    # W7
    wb[64:96, 2912:2928] = Wf[7].astype(np.float16)   # W8
    wb[96:112, 2928:2938] = Wf[8].astype(np.float16)  # W9
    wb1, wb = np.ascontiguousarray(wb[:, 0:256]), np.ascontiguousarray(wb[:, 256:])
    bb = np.zeros((128, 12), dtype=np.float32)
    bb[:, 0] = bs[0]
    bb[:, 1], bb[:, 2] = bs[1][0:128], bs[1][128:256]
    for c in range(4):
        bb[:, 3 + c] = bs[2][c * 128:(c + 1) * 128]
    bb[:, 7], bb[:, 8] = bs[3][0:128], bs[3][128:256]
    bb[:, 9] = bs[4]
    bb[0:64, 10], bb[64:96, 10], bb[96:112, 10] = bs[5], bs[6], bs[7]
    bb[0:10, 11] = bs[8]
    shared = {"WB1": wb1, "WB": wb, "BB": bb}

    in_maps = []
    for c in range(NCORES):
        sl = slice(c * SHARD, (c + 1) * SHARD)
        pack = np.zeros((NBLK, 128, NPACK, BLK), dtype=np.float16)
        xT = x[sl].T  # (256, SHARD)
        mT = [None] + [(ms[i][sl] != 0).T.astype(np.float16) for i in range(8)]
        for b in range(NBLK):
            cs = slice(b * BLK, (b + 1) * BLK)
            pack[b, :, 0, :] = xT[0:128, cs]
            pack[b, :, 1, :] = xT[128:256, cs]
            pack[b, :, 2, :] = mT[1][:, cs]
            pack[b, :, 3, :], pack[b, :, 4, :] = mT[2][0:128, cs], mT[2][128:256, cs]
            for k in range(4):
                pack[b, :, 5 + k, :] = mT[3][k * 128:(k + 1) * 128, cs]
            pack[b, :, 9, :], pack[b, :, 10, :] = mT[4][0:128, cs], mT[4][128:256, cs]
            pack[b, :, 11, :] = mT[5][:, cs]
            pack[b, 0:64, 12, :] = mT[6][:, cs]
            pack[b, 64:96, 12, :] = mT[7][:, cs]
            pack[b, 96:112, 12, :] = mT[8][:, cs]
        in_maps.append({"pack": pack, **shared})
    return in_maps


def kernel(**inputs) -> np.ndarray:
    from concourse.bass_utils import run_bass_kernel_spmd

    nc = _get_program()
    in_maps = _host_prep(inputs)
    res = run_bass_kernel_spmd(nc, in_maps, list(range(NCORES)))
    out = np.empty((BATCH, DIMS[-1]), dtype=np.float32)
    for c in range(NCORES):
        out[c * SHARD:(c + 1) * SHARD, :] = res.results[c]["outT"].T
    return out


# revision 11
# speedup vs baseline: 1.1552x; 1.1552x over previous
"""Trainium2 Bass kernel for the 9-layer dense MLP (dropout-mask training forward).

Strategy (pure data parallel, 8 cores, 8192 batch rows each):
  - Activations kept transposed on-chip: features on partitions, batch cols on free dim.
    Each layer computes zT = W^T @ hT via nc.tensor.matmul(out, lhsT=W, rhs=hT).
  - fp16 weights/activations/masks (fp32 PSUM accumulation), fp32 biases + output.
  - Dropout masks binarized on host ({0,1} fp16); the 1/keep scale is folded into the
    next layer's weights.
  - Host pack layout [NBLK, 128, NPACK, BLK] so each per-block DMA is contiguous per
    partition (2 DMAs per block: x+m1, then the remaining masks).
  - PSUM: one shared pool of 4x [128,1024] fp32 tiles (8 banks). Matmuls write 512-col
    halves; drains are single FD=1024 instructions (fused bias+relu) split ~3:1
    ACT:DVE; mask multiplies are FD=2048 DVE tensor_tensor with a measured dose
    offloaded to GpSimd.
  - Small layers 6/7/8 partition-packed (offsets 0/64/96 via matmul tile_position);
    each ladder step drains immediately to SBUF so its PSUM tile recycles fast.
    Block b's ladder is software-pipelined into block b+1's big-layer bursts.
  - A short burst of dummy matmuls at t~1us keeps the PE HAM activity monitor busy so
    the array is at full clock (K=8/8) when real work arrives.
"""

import sys

sys.path.insert(0, "/opt/trn_rl_repo")

import numpy as np

DIMS = [256, 128, 256, 512, 256, 128, 64, 32, 16, 10]
NCORES = 8
BATCH = 65536
SHARD = BATCH // NCORES  # 8192
MSUB = 512               # matmul N (PSUM bank limit for fp32)
DSUB = 1024              # drain granularity (2 banks)
BLK = 2048               # block columns
NBLK = SHARD // BLK      # 4
NSUB = BLK // MSUB       # 4
NDR = BLK // DSUB        # 2

# pack chunk layout (each chunk = 128 partitions x BLK cols, fp16), per block:
#   0,1: xT        2: m1        3,4: m2      5-8: m3
#   9,10: m4       11: m5       12: m6/m7/m8 partition-packed at rows 0/64/96
NPACK = 13

_PROG = {}


def _raise_sbuf_cap():
    # tile_utils.max_sbuf_usage is a stale 192KB constant; cayman has 208KB usable.
    import concourse.tile_utils as tu

    if getattr(tu, "max_sbuf_usage", 0) < 206 * 1024:
        tu.max_sbuf_usage = 206 * 1024


def _dedup_ldweights(nc):
    """Remove back-to-back redundant LDWEIGHTS (same stationary operand) so
    consecutive same-weight matmuls pipeline on the PE. Only drops LDW
    instructions that carry no semaphore waits/updates."""
    removed = 0
    for fn in nc.m.functions:
        for blk in fn.blocks:
            il = blk.instructions
            keep, last_sig = [], None
            for inst in il:
                nm = type(inst).__name__
                if nm == "InstLdweights":
                    sig = (str(inst.ins[0]), str(inst.is_transpose), str(inst.perf_mode),
                           str(getattr(inst, "tile_position", None)))
                    si = inst.sync_info
                    clean = si is None or (not si.on_wait and not si.on_update)
                    if sig == last_sig and clean:
                        removed += 1
                        continue
                    last_sig = sig
                keep.append(inst)
            if removed and len(keep) != len(il):
                while il:
                    il.pop()
                il.extend(keep)
    return removed


def _build_program():
    import concourse.bass as bass
    import concourse.tile as tile
    from concourse import bacc, mybir

    _raise_sbuf_cap()

    f16 = mybir.dt.float16
    f32 = mybir.dt.float32
    RELU = mybir.ActivationFunctionType.Relu
    IDENT = mybir.ActivationFunctionType.Identity
    ADD = mybir.AluOpType.add
    MAX = mybir.AluOpType.max

    nc = bacc.Bacc("TRN2", target_bir_lowering=False, debug=False, num_devices=NCORES)

    pack_d = nc.dram_tensor("pack", [NBLK, 128, NPACK, BLK], f16, kind="ExternalInput").ap()
    # weights in two host-laid-out fp16 blobs (W1 separate so the first
    # LDWEIGHTS isn't gated on the full blob), biases in one fp32 blob
    wb1_d = nc.dram_tensor("WB1", [128, 256], f16, kind="ExternalInput").ap()
    wb_d = nc.dram_tensor("WB", [128, 2688], f16, kind="ExternalInput").ap()
    bb_d = nc.dram_tensor("BB", [128, 12], f32, kind="ExternalInput").ap()
    out_d = nc.dram_tensor("outT", [10, SHARD], f32, kind="ExternalOutput").ap()

    with tile.TileContext(nc) as tc:
        with (
            tc.tile_pool(name="wpool", bufs=1) as wp,
            tc.tile_pool(name="mk", bufs=2) as mkp,
            tc.tile_pool(name="hr", bufs=1) as hrp,
            tc.tile_pool(name="osb", bufs=2) as outp,
            tc.tile_pool(name="ps", bufs=4, space="PSUM") as psp,
        ):
            wall = wp.tile([128, 2944], f16, tag="wall")
            ball = wp.tile([128, 12], f32, tag="ball")
            scratch = wp.tile([128, 512], f16, tag="scratch")
            # blob column offsets: w1@0(256) w2@256(256) w3@512(1024) w4@1536(1024)
            #   w5@2560(256) w6@2816(64) w789@2880(64: W7 r0-63 c0-31, W8 r64-95
            #   c32-47, W9 r96-111 c48-57)
            WOFF = {1: 0, 2: 256, 3: 512, 4: 1536, 5: 2560, 6: 2816, 789: 2880}
            w789 = wall[:, WOFF[789]:WOFF[789] + 64]
            b15 = ball[:, 0:10]
            b678 = ball[:, 10:11]
            b9 = ball[0:10, 11:12]

            def wslice(l, k, c, N):
                base = WOFF[l] + k * N
                return wall[:, base + c * 128: base + (c + 1) * 128]

            def drain_relu(eng, dst, zsrc, bias_ap):
                if eng == "act":
                    nc.scalar.activation(dst, zsrc, RELU, bias=bias_ap)
                else:
                    nc.vector.tensor_scalar(dst, zsrc, bias_ap, 0.0, ADD, MAX)

            def mask_mul(eng, dst, src, msrc):
                if eng == "gps":
                    nc.gpsimd.tensor_mul(dst, src, msrc)
                else:
                    nc.vector.tensor_mul(dst, src, msrc)

            # drain engine picker: ~70:30 act:dve (ACT is cheaper per element but
            # DVE has mask work too; this balances their queues)
            dr_i = [0]

            def pick_drain():
                i = dr_i[0]
                dr_i[0] += 1
                return "dve" if i % 10 in (2, 5, 8) else "act"

            state = {}
            packs = {}
            hrs = {}

            def issue_pack_dmas(b):
                # per-chunk tiles/DMAs: each mask tile's ring slot is released as
                # soon as its own layer consumes it, so block b+2's DMAs start
                # early instead of waiting for ALL of block b's masks (WAR).
                pk3 = mkp.tile([128, 3, BLK], f16, tag="pk3", name=f"pk3_{b}")
                m2t = mkp.tile([128, 2, BLK], f16, tag="m2", name=f"m2_{b}")
                m3t = mkp.tile([128, 4, BLK], f16, tag="m3", name=f"m3_{b}")
                m4t = mkp.tile([128, 2, BLK], f16, tag="m4", name=f"m4_{b}")
                m5t = mkp.tile([128, 1, BLK], f16, tag="m5", name=f"m5_{b}")
                # m678 is read by block b's ladder during block b+1, so bufs=3
                # keeps block b+2's DMA from waiting on the ladder (WAR).
                m678t = mkp.tile([128, 1, BLK], f16, tag="m678", name=f"m678_{b}",
                                 bufs=3)
                if b == 0:
                    nc.sync.dma_start(wall[:, 0:256], wb1_d[:])
                    nc.sync.dma_start(pk3[:], pack_d[0, :, 0:3, :])
                    nc.sync.dma_start(ball[:], bb_d[:])
                    nc.sync.dma_start(wall[:, 256:], wb_d[:])
                else:
                    nc.sync.dma_start(pk3[:], pack_d[b, :, 0:3, :])
                nc.sync.dma_start(m2t[:], pack_d[b, :, 3:5, :])
                nc.sync.dma_start(m3t[:], pack_d[b, :, 5:9, :])
                nc.sync.dma_start(m4t[:], pack_d[b, :, 9:11, :])
                nc.sync.dma_start(m5t[:], pack_d[b, :, 11:12, :])
                nc.sync.dma_start(m678t[:], pack_d[b, :, 12:13, :])
                packs[b] = (pk3, m2t, m3t, m4t, m5t, m678t)

            # (Kc, layer, wN, Cc, bias_off, hrtag)
            LAYER_CFG = [
                (2, 1, 128, 1, 0, "hr1"),
                (1, 2, 256, 2, 1, "hr2"),
                (2, 3, 512, 4, 3, "hr3"),
                (4, 4, 256, 2, 7, "hr4"),
                (2, 5, 128, 1, 9, "hr5"),
            ]
            # mask engine schedule per layer index. GpSimd is ~4x slower than DVE
            # per element, so it only gets masks with slack before their consumer:
            # m1 (L1 runs a block ahead) and m5 (ladder consumes it next block).
            # L2/L3/L4 masks sit on the next layer's critical path -> DVE only.
            MASK_MODE = {
                0: ["split"],
                1: ["dve", "dve"],
                2: ["dve", "dve", "dve", "dve"],
                3: ["dve", "dve"],
                4: ["split"],
            }

            def emit_layer(b, li):
                Kc, wl, wN, Cc, boff, hrtag = LAYER_CFG[li]
                hr = hrp.tile([128, Cc, BLK], f16, tag=hrtag, name=hrtag + f"_{b}",
                              bufs=2 if hrtag in ("hr5", "hr2", "hr1") else 1)
                pk3 = packs[b][0]
                hin = pk3 if li == 0 else hrs[(b, li - 1)]

                def msl(c, cols):
                    if li == 0:
                        return pk3[:, 2, cols]
                    return packs[b][li][:, c, cols]

                zs = {}
                for c in range(Cc):
                    for u in range(NDR):
                        zs[c, u] = psp.tile([128, DSUB], f32, tag="ps",
                                            name=f"z_{hrtag}_{b}_{c}_{u}")
                if b == 0 and li == 0:
                    # startup: u-outer so each 1024-col window flows MM -> drain
                    # -> mask as early as possible
                    for u in range(NDR):
                        for k in range(Kc):
                            for t in range(2):
                                nc.tensor.matmul(
                                    zs[0, u][:, bass.ts(t, MSUB)],
                                    wslice(wl, k, 0, wN),
                                    hin[:, k, u * DSUB + t * MSUB:
                                        u * DSUB + (t + 1) * MSUB],
                                    start=(k == 0), stop=(k == Kc - 1))
                        drain_relu("dve" if u == 0 else "act",
                                   hr[:, 0, bass.ts(u, DSUB)], zs[0, u][:],
                                   b15[:, 0:1])
                        mask_mul("dve", hr[:, 0, bass.ts(u, DSUB)],
                                 hr[:, 0, bass.ts(u, DSUB)],
                                 msl(0, bass.ts(u, DSUB)))
                    hrs[(b, li)] = hr
                    return
                # weight-major matmuls so consecutive MMs share one LDWEIGHTS
                for c in range(Cc):
                    for k in range(Kc):
                        wap = wslice(wl, k, c, wN)
                        for t in range(NSUB):
                            nc.tensor.matmul(
                                zs[c, t // 2][:, bass.ts(t % 2, MSUB)], wap,
                                hin[:, k, bass.ts(t, MSUB)],
                                start=(k == 0), stop=(k == Kc - 1))
                for u in range(NDR):
                    for c in range(Cc):
                        drain_relu(pick_drain(), hr[:, c, bass.ts(u, DSUB)],
                                   zs[c, u][:], b15[:, boff + c:boff + c + 1])
                full = slice(0, BLK)
                for c in range(Cc):
                    mode = MASK_MODE[li][c]
                    if mode == "dve":
                        mask_mul("dve", hr[:, c, full], hr[:, c, full], msl(c, full))
                    else:
                        hs0, hs1 = bass.ts(0, DSUB), bass.ts(1, DSUB)
                        mask_mul("dve", hr[:, c, hs0], hr[:, c, hs0], msl(c, hs0))
                        mask_mul("gps", hr[:, c, hs1], hr[:, c, hs1], msl(c, hs1))
                hrs[(b, li)] = hr
                if li > 0:
                    del hrs[(b, li - 1)]

            # --- small-layer ladder (L6/L7/L8), partition-packed -----------------
            # step s writes partition range p0:p1 of hr678; fresh PSUM tiles per
            # step, drained immediately so banks recycle.
            LAD_CFG = [
                ((0, 64), None, None),     # L6: full-K weight slice, no tile_position
                ((64, 96), (0, 64), (0, 64)),
                ((96, 112), (64, 96), (64, 96)),
            ]

            def emit_ladder_step(b, step, fine=False):
                st = state[b]
                hm5, m678 = st["hm5"], st["m678"]
                if step == 0:
                    st["hr678"] = hrp.tile([128, 1, BLK], f16, tag="hr678",
                                           name=f"hr678_{b}", bufs=2)
                hr678 = st["hr678"]
                (p0, p1), tile_pos, brange = LAD_CFG[step]
                if step == 0:
                    wap = wall[:, WOFF[6]:WOFF[6] + 64]
                elif step == 1:
                    wap = w789[0:64, 0:32]
                else:
                    wap = w789[64:96, 32:48]
                wins = range(NSUB) if fine else range(NDR)
                wsz = MSUB if fine else DSUB
                for u in wins:
                    zh = psp.tile([128, wsz], f32, tag="ps", name=f"zl_{b}_{step}_{u}")
                    nmm = 1 if fine else 2
                    for t in range(nmm):
                        rhs_sl = slice(u * wsz + t * MSUB, u * wsz + (t + 1) * MSUB)
                        rhs = (hm5[:, 0, rhs_sl] if step == 0 else
                               hr678[brange[0]:brange[1], 0, rhs_sl])
                        out_sl = zh[p0:p1, bass.ts(t, MSUB)] if not fine else zh[p0:p1, :]
                        if tile_pos is None:
                            nc.tensor.matmul(out_sl, wap, rhs, start=True, stop=True)
                        else:
                            nc.tensor.matmul(out_sl, wap, rhs, start=True, stop=True,
                                             tile_position=tile_pos)
                    dst = hr678[p0:p1, 0, u * wsz:(u + 1) * wsz]
                    drain_relu("dve" if u % 2 == 0 else "act",
                               dst, zh[p0:p1, :], b678[p0:p1, 0:1])
                    mask_mul("gps" if (step == 0 and not fine and u == 1) else "dve",
                             dst, dst, m678[p0:p1, 0, u * wsz:(u + 1) * wsz])

            def emit_l9(b, fine=False):
                st = state[b]
                hm678 = st["hr678"]
                osb = outp.tile([10, BLK], f32, tag="osb", bufs=2, name=f"osb_{b}")
                wins = range(NSUB) if fine else range(NDR)
                wsz = MSUB if fine else DSUB
                for u in wins:
                    z9 = psp.tile([128, wsz], f32, tag="ps", name=f"z9_{b}_{u}")
                    nmm = 1 if fine else 2
                    for t in range(nmm):
                        rhs_sl = slice(u * wsz + t * MSUB, u * wsz + (t + 1) * MSUB)
                        nc.tensor.matmul(z9[0:10, bass.ts(t, MSUB)] if not fine
                                         else z9[0:10, :],
                                         w789[96:112, 48:58],
                                         hm678[96:112, 0, rhs_sl],
                                         start=True, stop=True, tile_position=(96, 0))
                    nc.scalar.activation(osb[:, u * wsz:(u + 1) * wsz], z9[0:10, :],
                                         IDENT, bias=b9[:, 0:1])
                nc.sync.dma_start(out_d[:, bass.ts(b, BLK)], osb[:])
                del state[b]

            # --- schedule ---------------------------------------------------------
            # PE warmup: dummy matmuls on a memset scratch tile so the HAM clock
            # gate opens before real work arrives (weights/x still in DMA).
            nc.vector.memset(scratch[:], 0.0)
            zw = psp.tile([128, MSUB], f32, tag="ps", name="zwarm")
            for i in range(16):
                nc.tensor.matmul(zw[:], scratch[:, 0:128], scratch[:], start=True,
                                 stop=True)

            issue_pack_dmas(0)
            emit_layer(0, 0)                       # L1(0) during startup
            for b in range(NBLK):
                if b + 1 < NBLK:
                    issue_pack_dmas(b + 1)
                emit_layer(b, 1)                   # L2
                if b > 0 and (b - 1) in state:
                    emit_ladder_step(b - 1, 0)     # L6(b-1)
                if b > 1 and (b - 2) in state:
                    emit_l9(b - 2)                 # L9(b-2)
                emit_layer(b, 2)                   # L3
                if b > 0 and (b - 1) in state:
                    emit_ladder_step(b - 1, 1)     # L7(b-1)
                emit_layer(b, 3)                   # L4
                if b + 1 < NBLK:
                    emit_layer(b + 1, 0)           # L1(b+1) pipelined ahead
                if b > 0 and (b - 1) in state:
                    emit_ladder_step(b - 1, 2)     # L8(b-1)
                emit_layer(b, 4)                   # L5
                state[b] = {"hm5": hrs.pop((b, 4)), "m678": packs[b][5]}

            # tail: last block's ladder at fine (512) granularity to shorten the
            # exposed serial chain; block NBLK-2's pending L9 fills the first gap
            emit_ladder_step(NBLK - 1, 0, fine=True)
            if (NBLK - 2) in state:
                emit_l9(NBLK - 2)
            emit_ladder_step(NBLK - 1, 1, fine=True)
            emit_ladder_step(NBLK - 1, 2, fine=True)
            emit_l9(NBLK - 1, fine=True)

    _dedup_ldweights(nc)
    nc.compile()
    return nc


def _get_program():
    if "nc" not in _PROG:
        _PROG["nc"] = _build_program()
    return _PROG["nc"]


def _host_prep(inputs):
    """Build per-core input maps (numpy only)."""
    x = np.asarray(inputs["x"], dtype=np.float32)
    Ws = [np.asarray(inputs[f"W{i}"], dtype=np.float32) for i in range(1, 10)]
    bs = [np.asarray(inputs[f"b{i}"], dtype=np.float32) for i in range(1, 10)]
    ms = [np.asarray(inputs[f"m{i}"], dtype=np.float32) for i in range(1, 9)]

    # fold dropout scale into next layer's weights; binarize masks
    Wf = [Ws[0]]
    for i in range(1, 9):
        s = float(ms[i - 1].max())
        if s <= 0.0:  # degenerate all-dropped mask; keep weights unscaled
            s = 1.0
        Wf.append(Ws[i] * np.float32(s))

    # weight blob: w1@0 w2@256 w3@512 w4@1536 w5@2560 w6@2816 w789@2880
    WOFF = {1: 0, 2: 256, 3: 512, 4: 1536, 5: 2560, 6: 2816, 789: 2880}
    wb = np.zeros((128, 2944), dtype=np.float16)
    for l in range(1, 7):
        W = Wf[l - 1]
        K, N = W.shape
        for k in range((K + 127) // 128):
            blk = W[k * 128:(k + 1) * 128].astype(np.float16)
            wb[: blk.shape[0], WOFF[l] + k * N: WOFF[l] + k * N + N] = blk
    wb[0:64, 2880:2912] = Wf[6].astype(np.float16)    # W7
    wb[64:96, 2912:2928] = Wf[7].astype(np.float16)   # W8
    wb[96:112, 2928:2938] = Wf[8].astype(np.float16)  # W9
    wb1, wb = np.ascontiguousarray(wb[:, 0:256]), np.ascontiguousarray(wb[:, 256:])
    bb = np.zeros((128, 12), dtype=np.float32)
    bb[:, 0] = bs[0]
    bb[:, 1], bb[:, 2] = bs[1][0:128], bs[1][128:256]
    for c in range(4):
        bb[:, 3 + c] = bs[2][c * 128:(c + 1) * 128]
    bb[:, 7], bb[:, 8] = bs[3][0:128], bs[3][128:256]
    bb[:, 9] = bs[4]
    bb[0:64, 10], bb[64:96, 10], bb[96:112, 10] = bs[5], bs[6], bs[7]
    bb[0:10, 11] = bs[8]
    shared = {"WB1": wb1, "WB": wb, "BB": bb}

    in_maps = []
    for c in range(NCORES):
        sl = slice(c * SHARD, (c + 1) * SHARD)
        pack = np.zeros((NBLK, 128, NPACK, BLK), dtype=np.float16)
        xT = x[sl].T  # (256, SHARD)
        mT = [None] + [(ms[i][sl] != 0).T.astype(np.float16) for i in range(8)]
        for b in range(NBLK):
            cs = slice(b * BLK, (b + 1) * BLK)
            pack[b, :, 0, :] = xT[0:128, cs]
            pack[b, :, 1, :] = xT[128:256, cs]
            pack[b, :, 2, :] = mT[1][:, cs]
            pack[b, :, 3, :], pack[b, :, 4, :] = mT[2][0:128, cs], mT[2][128:256, cs]
            for k in range(4):
                pack[b, :, 5 + k, :] = mT[3][k * 128:(k + 1) * 128, cs]
            pack[b, :, 9, :], pack[b, :, 10, :] = mT[4][0:128, cs], mT[4][128:256, cs]
            pack[b, :, 11, :] = mT[5][:, cs]
            pack[b, 0:64, 12, :] = mT[6][:, cs]
            pack[b, 64:96, 12, :] = mT[7][:, cs]
            pack[b, 96:112, 12, :] = mT[8][:, cs]
        in_maps.append({"pack": pack, **shared})
    return in_maps


def kernel(**inputs) -> np.ndarray:
    from concourse.bass_utils import run_bass_kernel_spmd

    nc = _get_program()
    in_maps = _host_prep(inputs)
    res = run_bass_kernel_spmd(nc, in_maps, list(range(NCORES)))
    out = np.empty((BATCH, DIMS[-1]), dtype=np.float32)
    for c in range(NCORES):
        out[c * SHARD:(c + 1) * SHARD, :] = res.results[c]["outT"].T
    return out
